# revision 17
# baseline (speedup 1.0000x reference)
"""Trainium2 Bass kernel for nn_Conv2DExperimental (MVN-sampled 3x3 conv).

Computation (per the nn.Module):
  L    = tril(weight_L, -1) + softplus(diag(weight_L)) * I      # [O,I,D,D], D=9
  w    = weight_loc + einsum('oiab,oib->oia', L, eps_w)         # [O,I,3,3]
  b    = bias_loc + eps_b * softplus(bias_ro)                   # [O]
  out  = conv2d(x, w, SAME, NCHW) + b
All of the model math (softplus, tril matvec, bias sampling, conv, bias add)
runs on device; the host only reshapes / masks / pads / converts dtypes.

Distribution: data-parallel over the batch dim of x (32 images -> 8 cores x 4),
with the weight sampling replicated on every core (it is tiny).

Per-core kernel (bf16 data path, fp32 PSUM accumulate):
  - x is zero-padded to 226x226 and converted to bf16 on the host, so every
    input strip is one fully-contiguous DMA with 13.5KB descriptors and no
    halo memsets exist. Strips load as two vertically-split tiles whose
    pool recycling staggers the two DMA bursts ~half a strip apart.
  - the conv runs as 4 concurrent 64x64 tile_position matmuls per tap
    (2 images on the PE row halves x 2 spatial 4-row blocks on the column
    halves), using the full 128x128 array on useful work. A group covers 8
    output rows of both images: per tap, each quadrant issues 2 matmuls
    (row-pairs) into 2 PSUM banks, so weights load once per 2 matmuls and
    each SBUF-out partition holds 4 contiguous output rows -> 1792B store
    descriptors (descriptor rate, not bytes, bounds the store stream).
  - ScalarE/VectorE evacuate PSUM with the bias add fused, emitting bf16;
    the host converts the bf16 output back to fp32 (untimed).
  - weight sampling: wL arrives host-masked (tril -1) with the diagonal as
    a separate compact tensor, so the tril matvec is one broadcast multiply
    plus one innermost-axis reduction on VectorE - no serial per-column
    chains (each dependent DVE op pays ~700ns semaphore latency). GpSimdE
    measured 4-7x slower per element, so it only does memsets.
  - wsamp is duplicated in the free dim so each PE transpose emits the tap
    matrix on both partition halves (for the two row-tile groups) at once.
"""

import sys
from contextlib import ExitStack

for _p in ("/opt/trn_rl_repo",):
    if _p not in sys.path:
        sys.path.insert(0, _p)

import numpy as np

import concourse.bass as bass
import concourse.bacc as bacc
import concourse.mybir as mybir
from concourse.tile import TileContext

F32 = mybir.dt.float32
BF16 = mybir.dt.bfloat16
AF = mybir.ActivationFunctionType

N_CORES = 8
O = 64
I = 64
KK = 3
D = KK * KK  # 9


def build_nc(nb=4, hh=224, ww=224, rstrip=32, o_bufs=3):
    assert nb % 2 == 0 and hh % rstrip == 0 and rstrip % 8 == 0
    wpad = ww + 2
    nstrips = hh // rstrip
    xrows = rstrip // 2 + 2  # rows per split x tile (A: top half, B: bottom)

    nc = bacc.Bacc("TRN2", target_bir_lowering=False, debug=False)

    x_t = nc.dram_tensor("x", [nb, I, hh + 2, wpad], BF16, kind="ExternalInput").ap()
    wl_t = nc.dram_tensor("wL", [O, I * D * D], BF16, kind="ExternalInput").ap()
    # smalls packs (wdiag | epsw | wloc) side by side, one DMA
    smalls_t = nc.dram_tensor("smalls", [O, 3 * I * D], BF16, kind="ExternalInput").ap()
    ident_t = nc.dram_tensor("ident", [O, O], F32, kind="ExternalInput").ap()
    bias3_t = nc.dram_tensor("bias3", [3, 128], F32, kind="ExternalInput").ap()
    out_t = nc.dram_tensor("out", [nb, O, hh, ww], BF16, kind="ExternalOutput").ap()

    with TileContext(nc) as tc, ExitStack() as stack:
        # ---------------- weight + bias sampling (one-time prologue) --------
        cp = stack.enter_context(tc.tile_pool(name="consts", bufs=1))
        wl = cp.tile([O, I * D * D], BF16, name="wl", tag="wl")
        smalls = cp.tile([O, 3 * I * D], BF16, name="smalls", tag="smalls")
        ident = cp.tile([O, O], F32, name="ident_s", tag="ident_s")
        b3 = cp.tile([128, 3], F32, name="b3", tag="b3")
        sp = cp.tile([O, I * D], F32, name="sp", tag="sp")
        tmp2 = cp.tile([O, I * D * D], BF16, name="tmp2", tag="tmp2")
        redt = cp.tile([O, I * D], F32, name="redt", tag="redt")
        # wsamp duplicated back-to-back so a transpose lhsT can span both
        # copies with a single [D, 128]-stride free dim
        wsamp = cp.tile([O, 2 * I * D], F32, name="wsamp", tag="wsamp")
        bias = cp.tile([128, 1], F32, name="bias", tag="bias")
        wts = cp.tile([128, D * O], BF16, name="wts", tag="wts")
        sp_b = cp.tile([128, 1], F32, name="sp_b", tag="sp_b")
        b3p = cp.tile([3, 128], F32, name="b3p", tag="b3p")

        wdiag = smalls[:, 0 : I * D]
        epsw = smalls[:, I * D : 2 * I * D]
        wloc = smalls[:, 2 * I * D : 3 * I * D]

        # prologue DMAs first on the sync queue (same queue as the x strip
        # loads: first-queued wins the DMA engines); wL is split in two so
        # the tril pipeline can start on the first half early
        nc.sync.dma_start(b3p[:], bias3_t[:])
        nc.sync.dma_start(smalls[:], smalls_t[:])
        H2 = I * D * D // 2
        nc.sync.dma_start(wl[:, 0:H2], wl_t[:, 0:H2])
        nc.sync.dma_start(wl[:, H2:], wl_t[:, H2:])
        nc.sync.dma_start(ident[:], ident_t[:])

        # PE warm-up feed: zero tiles via GpSimd (no input deps)
        identr = cp.tile([O, O], BF16, name="identr", tag="identr")
        junk = cp.tile([O, 256], BF16, name="junk", tag="junk")
        with tc.high_priority():
            nc.gpsimd.memset(identr[:], 0.0)
            nc.gpsimd.memset(junk[:], 0.0)

        with tc.tile_pool(name="wp", bufs=1, space="PSUM") as wp:
            # HAM warm-up: bridge PE activity from kernel entry to the tap
            # transposes; sized to end near sampling-ready (~210ns each).
            warm = wp.tile([O, 256], F32, name="warm")
            n_warm = 75
            for k in range(n_warm):
                nc.tensor.matmul(
                    warm[:], identr[:], junk[:],
                    start=(k == 0), stop=(k == n_warm - 1),
                )

        # softplus(x) = ln(exp(x) + 1): no Softplus LUT in this toolchain.
        with tc.high_priority():
            nc.scalar.activation(sp[:], wdiag, AF.Exp)
            nc.scalar.activation(sp[:], sp[:], AF.Ln, bias=1.0)

        # wsamp = wloc + softplus(diag) * eps + (tril(wL,-1) @ eps), all on
        # VectorE as a handful of large ops. prod+reduce are emitted first:
        # they gate the transposes and only need the wL/eps DMAs.
        sp3 = sp[:].rearrange("o (i a) -> o i a", i=I)
        e3 = epsw.rearrange("o (i a) -> o i a", i=I)
        l3 = wloc.rearrange("o (i a) -> o i a", i=I)
        w0 = wsamp[:, 0 : I * D].rearrange("o (i a) -> o i a", i=I)
        wl3 = wl[:].rearrange("o (i a b) -> o i a b", i=I, a=D)
        p3 = tmp2[:].rearrange("o (i a b) -> o i a b", i=I, a=D)
        eb = bass.AP(
            tensor=epsw.tensor,
            offset=epsw.offset,
            ap=[list(p) for p in epsw.ap[:1]] + [[D, I], [0, D], [1, D]],
        )
        # i-halves pipelined: prod/reduce of half 1 run while wL half 2
        # is still in flight
        def half(v, k, idim=1):
            ap = [list(p) for p in v.ap]
            st = ap[idim][0]
            ap[idim] = [st, I // 2]
            return bass.AP(tensor=v.tensor, offset=v.offset + k * (I // 2) * st, ap=ap)

        r3 = redt[:].rearrange("o (i a) -> o i a", i=I)
        for k in range(2):
            nc.vector.tensor_tensor(
                half(p3, k), half(wl3, k), half(eb, k), mybir.AluOpType.mult
            )
            nc.vector.tensor_reduce(
                half(r3, k), half(p3, k), mybir.AxisListType.X, mybir.AluOpType.add
            )
        nc.vector.tensor_tensor(w0, sp3, e3, mybir.AluOpType.mult)
        nc.vector.tensor_add(w0, w0, l3)
        nc.vector.tensor_add(wsamp[:, 0 : I * D], wsamp[:, 0 : I * D], redt[:])
        # duplicate wsamp so transposes can address (half, i) as one run
        nc.vector.tensor_copy(wsamp[:, I * D : 2 * I * D], wsamp[:, 0 : I * D])

        # transpose the 9 taps on the PE into T_a[ich, och] on BOTH partition
        # halves at once (lhsT free dim = 128 spanning the two wsamp copies),
        # packed 5 + 4 into two PSUM banks, then bf16-convert into wts.
        with tc.tile_pool(name="pt", bufs=1, space="PSUM") as ptp:
            ptA = ptp.tile([128, 5 * O], F32, name="ptA")
            ptB = ptp.tile([128, 4 * O], F32, name="ptB")
            for a in range(D):
                w_a = bass.AP(
                    tensor=wsamp[:].tensor,
                    offset=wsamp[:].offset + a,
                    ap=[list(p) for p in wsamp[:].ap[:1]] + [[D, 2 * I]],
                )
                dst_pt = ptA if a < 5 else ptB
                c = a if a < 5 else a - 5
                nc.tensor.matmul(
                    dst_pt[:, c * O : (c + 1) * O],
                    w_a,
                    ident[:],
                    is_transpose=True,
                    start=(c == 0),
                    stop=(c == (4 if a < 5 else 3)),
                    skip_group_check=True,
                )
            nc.vector.tensor_copy(wts[:, 0 : 5 * O], ptA[:])
            nc.vector.tensor_copy(wts[:, 5 * O : 9 * O], ptB[:])

            # bias path, off the conv critical path: bias3 arrives [3, 128]
            # (och duplicated on host); transpose to [128, 3], then
            # bias = loc + eps * softplus(ro). Needed by the first evac
            # (conv start + ~3.4us), not the first matmul.
            bp_ps = ptp.tile([128, 3], F32, name="bp_ps")
            nc.tensor.matmul(
                bp_ps[:], b3p[:], ident[0:3, 0:3], start=True, stop=True
            )
            nc.scalar.activation(b3[:], bp_ps[:], AF.Identity)
            nc.scalar.activation(sp_b[:], b3[:, 1:2], AF.Exp)
            nc.scalar.activation(sp_b[:], sp_b[:], AF.Ln, bias=1.0)
            nc.gpsimd.tensor_mul(sp_b[:], sp_b[:], b3[:, 2:3])
            nc.gpsimd.tensor_add(bias[:], b3[:, 0:1], sp_b[:])

        # ---------------- convolution ---------------------------------------
        # Group = 8 output rows of both images. 4 concurrent 64x64 PE tiles:
        # PE rows half = image, PE cols half = 4-row block parity. Per tap
        # each quadrant runs 2 row-pair matmuls into 2 PSUM banks:
        #   imgA rows 8j+0..1 -> bk0[0:64]   rows 8j+2..3 -> bk1[0:64]
        #   imgA rows 8j+4..5 -> bk0[64:]    rows 8j+6..7 -> bk1[64:]
        #   imgB rows 8j+0..1 -> bk2[0:64]   ... etc (banks 2,3)
        # SBUF out strip: partitions 0:64 = rows 8j+0..3, 64:128 = 8j+4..7;
        # free = [img, group, 4*ww]. Output DMA splits (partition half, img)
        # with 4-row (1792B) descriptors.
        xpa = stack.enter_context(tc.tile_pool(name="xa", bufs=3))
        xpb = stack.enter_context(tc.tile_pool(name="xb", bufs=3))
        op = stack.enter_context(tc.tile_pool(name="ostrip", bufs=o_bufs))
        pp = stack.enter_context(tc.tile_pool(name="acc", bufs=2, space="PSUM"))
        allstrips = []
        for pair in range(nb // 2):
            n0 = 2 * pair
            strips = [(s * rstrip, rstrip) for s in range(nstrips)]
            if pair == nb // 2 - 1 and rstrip >= 16:
                # Taper: split the final strip in half so the kernel does
                # not end waiting on one full-size store.
                h_last = strips.pop()[0]
                strips.append((h_last, rstrip // 2))
                strips.append((h_last + rstrip // 2, rstrip // 2))
            allstrips += [(n0, h0, rout) for h0, rout in strips]

        loaded = {}

        def emit_load(idx):
            n0_, h0_, rout_ = allstrips[idx]
            ng_ = rout_ // 8
            hr_ = rout_ // 2 + 2 if ng_ > 1 else rout_ + 2
            xa = xpa.tile([128, xrows, wpad], BF16, name="xs_a")
            src_ = x_t[n0_ : n0_ + 2, :, h0_ : h0_ + hr_, :].rearrange(
                "n i h w -> (n i) h w"
            )
            nc.sync.dma_start(xa[:, 0:hr_, :], src_)
            xb = None
            if ng_ > 1:
                xb = xpb.tile([128, xrows, wpad], BF16, name="xs_b")
                srcb_ = x_t[
                    n0_ : n0_ + 2, :, h0_ + rout_ // 2 : h0_ + rout_ + 2, :
                ].rearrange("n i h w -> (n i) h w")
                nc.sync.dma_start(xb[:, 0 : rout_ // 2 + 2, :], srcb_)
            loaded[idx] = (xa, xb)

        # loads run 2 strips ahead and are emitted BEFORE the previous
        # strip's stores, so store triggers never head-of-line-block them
        emit_load(0)
        if len(allstrips) > 1:
            emit_load(1)
        for si, (n0, h0, rout) in enumerate(allstrips):
            ngroups = rout // 8
            xs_a, xs_b = loaded.pop(si)
            os_ = op.tile([128, rout * ww], BF16, name="os_")
            for j in range(ngroups):
                bks = [pp.tile([128, 2 * ww], F32, name=f"bk{k}") for k in range(4)]
                if 8 * j >= rout // 2 and ngroups > 1:
                    xs, rbase = xs_b, 8 * j - rout // 2
                else:
                    xs, rbase = xs_a, 8 * j
                for tap in range(D):
                    dy, dx = tap // 3 - 1, tap % 3 - 1
                    st, sp_ = (tap == 0), (tap == D - 1)
                    w_lo = wts[0:O, tap * O : (tap + 1) * O]
                    w_hi = wts[O:128, tap * O : (tap + 1) * O]
                    for rp in range(2):
                        rhs = []
                        for par in range(2):
                            rr = rbase + 4 * par + 2 * rp
                            off = (rr + 1 + dy) * wpad + 1 + dx
                            for half in range(2):
                                base = xs[64 * half : 64 * half + 64]
                                rhs.append(
                                    bass.AP(
                                        tensor=base.tensor,
                                        offset=base.offset + off,
                                        ap=[list(p) for p in base.ap[:1]]
                                        + [[wpad, 2], [1, ww]],
                                    )
                                )
                        # rhs order: [imgA par0, imgB par0, imgA par1, imgB par1]
                        nc.tensor.matmul(
                            bks[rp][0:O], w_lo, rhs[0],
                            start=st, stop=sp_, skip_group_check=True,
                        )
                        nc.tensor.matmul(
                            bks[rp][O:128], w_lo, rhs[2],
                            start=st, stop=sp_, skip_group_check=True,
                        )
                        nc.tensor.matmul(
                            bks[2 + rp][0:O], w_hi, rhs[1],
                            start=st, stop=sp_, skip_group_check=True,
                        )
                        nc.tensor.matmul(
                            bks[2 + rp][O:128], w_hi, rhs[3],
                            start=st, stop=sp_, skip_group_check=True,
                        )
                # evacuate with fused bias add: imgA banks on ScalarE,
                # imgB banks on VectorE; each bank's halves land at the
                # same free offset (parities live on partition halves)
                for rp in range(2):
                    sA = (j * 4 + 2 * rp) * ww
                    nc.scalar.activation(
                        os_[:, sA : sA + 2 * ww],
                        bks[rp][:], AF.Identity, bias=bias[:, 0:1],
                    )
                    sB = ((ngroups + j) * 4 + 2 * rp) * ww
                    nc.vector.tensor_scalar_add(
                        os_[:, sB : sB + 2 * ww],
                        bks[2 + rp][:], bias[:, 0:1],
                    )
            if si + 2 < len(allstrips):
                emit_load(si + 2)
            # store DMAs: (partition half, image), 4-row descriptors. The
            # final two strips store per group so the kernel does not end
            # draining a whole strip of descriptor-rate-bound stores.
            per_group = si >= len(allstrips) - 2
            for jg in range(ngroups) if per_group else [None]:
                for img in range(2):
                    for par in range(2):
                        os_h = os_[64 * par : 64 * par + 64]
                        j0 = jg if per_group else 0
                        ng = 1 if per_group else ngroups
                        src_os = bass.AP(
                            tensor=os_h.tensor,
                            offset=os_h.offset
                            + (img * ngroups + j0) * 4 * ww,
                            ap=[list(p) for p in os_h.ap[:1]]
                            + [[4 * ww, ng], [1, 4 * ww]],
                        )
                        dst = bass.AP(
                            tensor=out_t.tensor,
                            offset=out_t.offset
                            + (n0 + img) * O * hh * ww
                            + (h0 + 8 * j0 + 4 * par) * ww,
                            ap=[[hh * ww, O], [8 * ww, ng], [1, 4 * ww]],
                        )
                        q = nc.gpsimd if par == 0 else nc.sync
                        q.dma_start(dst, src_os)

    nc.compile()
    return nc


_CACHED_NC = None


def _host_inputs(x_shard_padded, weight_loc, weight_L, bias_loc, bias_ro, eps_w, eps_b):
    import ml_dtypes

    bf = ml_dtypes.bfloat16
    wdiag = np.diagonal(weight_L, axis1=-2, axis2=-1).reshape(O, I * D)
    smalls = np.concatenate(
        [wdiag, eps_w.reshape(O, I * D), weight_loc.reshape(O, I * D)], axis=1
    )
    return {
        "x": x_shard_padded,
        "wL": np.ascontiguousarray(
            np.tril(weight_L, -1).reshape(O, I * D * D).astype(bf)
        ),
        "smalls": np.ascontiguousarray(smalls.astype(bf)),
        "ident": np.eye(O, dtype=np.float32),
        "bias3": np.ascontiguousarray(
            np.tile(np.stack([bias_loc, bias_ro, eps_b]).astype(np.float32), (1, 2))
        ),
    }


def _pad_x(x):
    """fp32 [N, I, H, W] -> bf16 [N, I, H+2, W+2] zero-padded halo."""
    import ml_dtypes

    n, i, h, w = x.shape
    xp = np.zeros((n, i, h + 2, w + 2), dtype=ml_dtypes.bfloat16)
    xp[:, :, 1 : h + 1, 1 : w + 1] = x.astype(ml_dtypes.bfloat16)
    return xp


def kernel(x, weight_loc, weight_L, bias_loc, bias_ro, eps_w, eps_b):
    global _CACHED_NC
    from concourse.bass_utils import run_bass_kernel_spmd

    x = np.asarray(x, np.float32)
    nb = x.shape[0] // N_CORES
    if _CACHED_NC is None:
        _CACHED_NC = build_nc(nb=nb)
    nc = _CACHED_NC

    xpad = _pad_x(x)
    in_maps = [
        _host_inputs(
            xpad[c * nb : (c + 1) * nb],
            np.asarray(weight_loc),
            np.asarray(weight_L),
            np.asarray(bias_loc),
            np.asarray(bias_ro),
            np.asarray(eps_w),
            np.asarray(eps_b),
        )
        for c in range(N_CORES)
    ]
    res = run_bass_kernel_spmd(nc, in_maps, list(range(N_CORES)))
    return np.concatenate(
        [res.results[c]["out"].astype(np.float32) for c in range(N_CORES)], axis=0
    )


# revision 18
# speedup vs baseline: 1.0026x; 1.0026x over previous
"""Trainium2 Bass kernel for nn_Conv2DExperimental (MVN-sampled 3x3 conv).

Computation (per the nn.Module):
  L    = tril(weight_L, -1) + softplus(diag(weight_L)) * I      # [O,I,D,D], D=9
  w    = weight_loc + einsum('oiab,oib->oia', L, eps_w)         # [O,I,3,3]
  b    = bias_loc + eps_b * softplus(bias_ro)                   # [O]
  out  = conv2d(x, w, SAME, NCHW) + b
All of the model math (softplus, tril matvec, bias sampling, conv, bias add)
runs on device; the host only reshapes / masks / pads / converts dtypes.

Distribution: data-parallel over the batch dim of x (32 images -> 8 cores x 4),
with the weight sampling replicated on every core (it is tiny).

Per-core kernel (bf16 data path, fp32 PSUM accumulate):
  - x is zero-padded to 226x226 and converted to bf16 on the host, so every
    input strip is one fully-contiguous DMA with 13.5KB descriptors and no
    halo memsets exist. Strips load as two vertically-split tiles whose
    pool recycling staggers the two DMA bursts ~half a strip apart.
  - the conv runs as 4 concurrent 64x64 tile_position matmuls per tap
    (2 images on the PE row halves x 2 spatial 4-row blocks on the column
    halves), using the full 128x128 array on useful work. A group covers 8
    output rows of both images: per tap, each quadrant issues 2 matmuls
    (row-pairs) into 2 PSUM banks, so weights load once per 2 matmuls and
    each SBUF-out partition holds 4 contiguous output rows -> 1792B store
    descriptors (descriptor rate, not bytes, bounds the store stream).
  - ScalarE/VectorE evacuate PSUM with the bias add fused, emitting bf16;
    the host converts the bf16 output back to fp32 (untimed).
  - weight sampling: wL arrives host-masked (tril -1) with the diagonal as
    a separate compact tensor, so the tril matvec is one broadcast multiply
    plus one innermost-axis reduction on VectorE - no serial per-column
    chains (each dependent DVE op pays ~700ns semaphore latency). GpSimdE
    measured 4-7x slower per element, so it only does memsets.
  - wsamp is duplicated in the free dim so each PE transpose emits the tap
    matrix on both partition halves (for the two row-tile groups) at once.
"""

import sys
from contextlib import ExitStack

for _p in ("/opt/trn_rl_repo",):
    if _p not in sys.path:
        sys.path.insert(0, _p)

import numpy as np

import concourse.bass as bass
import concourse.bacc as bacc
import concourse.mybir as mybir
from concourse.tile import TileContext

F32 = mybir.dt.float32
BF16 = mybir.dt.bfloat16
AF = mybir.ActivationFunctionType

N_CORES = 8
O = 64
I = 64
KK = 3
D = KK * KK  # 9


def build_nc(nb=4, hh=224, ww=224, rstrip=32, o_bufs=4):
    assert nb % 2 == 0 and hh % rstrip == 0 and rstrip % 8 == 0
    wpad = ww + 2
    nstrips = hh // rstrip
    xrows = rstrip // 2 + 2  # rows per split x tile (A: top half, B: bottom)

    nc = bacc.Bacc("TRN2", target_bir_lowering=False, debug=False)

    x_t = nc.dram_tensor("x", [nb, I, hh + 2, wpad], BF16, kind="ExternalInput").ap()
    wl_t = nc.dram_tensor("wL", [O, I * D * D], BF16, kind="ExternalInput").ap()
    # smalls packs (wdiag | epsw | wloc) side by side, one DMA
    smalls_t = nc.dram_tensor("smalls", [O, 3 * I * D], BF16, kind="ExternalInput").ap()
    ident_t = nc.dram_tensor("ident", [O, O], F32, kind="ExternalInput").ap()
    bias3_t = nc.dram_tensor("bias3", [3, 128], F32, kind="ExternalInput").ap()
    out_t = nc.dram_tensor("out", [nb, O, hh, ww], BF16, kind="ExternalOutput").ap()

    with TileContext(nc) as tc, ExitStack() as stack:
        # ---------------- weight + bias sampling (one-time prologue) --------
        cp = stack.enter_context(tc.tile_pool(name="consts", bufs=1))
        wl = cp.tile([O, I * D * D], BF16, name="wl", tag="wl")
        smalls = cp.tile([O, 3 * I * D], BF16, name="smalls", tag="smalls")
        ident = cp.tile([O, O], F32, name="ident_s", tag="ident_s")
        b3 = cp.tile([128, 3], F32, name="b3", tag="b3")
        sp = cp.tile([O, I * D], F32, name="sp", tag="sp")
        tmp2 = cp.tile([O, I * D * D], BF16, name="tmp2", tag="tmp2")
        redt = cp.tile([O, I * D], F32, name="redt", tag="redt")
        # wsamp duplicated back-to-back so a transpose lhsT can span both
        # copies with a single [D, 128]-stride free dim
        wsamp = cp.tile([O, 2 * I * D], F32, name="wsamp", tag="wsamp")
        bias = cp.tile([128, 1], F32, name="bias", tag="bias")
        wts = cp.tile([128, D * O], BF16, name="wts", tag="wts")
        sp_b = cp.tile([128, 1], F32, name="sp_b", tag="sp_b")
        b3p = cp.tile([3, 128], F32, name="b3p", tag="b3p")

        wdiag = smalls[:, 0 : I * D]
        epsw = smalls[:, I * D : 2 * I * D]
        wloc = smalls[:, 2 * I * D : 3 * I * D]

        # prologue DMAs first on the sync queue (same queue as the x strip
        # loads: first-queued wins the DMA engines); wL is split in two so
        # the tril pipeline can start on the first half early
        nc.sync.dma_start(b3p[:], bias3_t[:])
        nc.sync.dma_start(smalls[:], smalls_t[:])
        H2 = I * D * D // 2
        nc.sync.dma_start(wl[:, 0:H2], wl_t[:, 0:H2])
        nc.sync.dma_start(wl[:, H2:], wl_t[:, H2:])
        nc.sync.dma_start(ident[:], ident_t[:])

        # PE warm-up feed: zero tiles via GpSimd (no input deps)
        identr = cp.tile([O, O], BF16, name="identr", tag="identr")
        junk = cp.tile([O, 256], BF16, name="junk", tag="junk")
        with tc.high_priority():
            nc.gpsimd.memset(identr[:], 0.0)
            nc.gpsimd.memset(junk[:], 0.0)

        with tc.tile_pool(name="wp", bufs=1, space="PSUM") as wp:
            # HAM warm-up: bridge PE activity from kernel entry to the tap
            # transposes; sized to end near sampling-ready (~210ns each).
            warm = wp.tile([O, 256], F32, name="warm")
            n_warm = 60
            for k in range(n_warm):
                nc.tensor.matmul(
                    warm[:], identr[:], junk[:],
                    start=(k == 0), stop=(k == n_warm - 1),
                )

        # softplus(x) = ln(exp(x) + 1): no Softplus LUT in this toolchain.
        with tc.high_priority():
            nc.scalar.activation(sp[:], wdiag, AF.Exp)
            nc.scalar.activation(sp[:], sp[:], AF.Ln, bias=1.0)

        # wsamp = wloc + softplus(diag) * eps + (tril(wL,-1) @ eps), all on
        # VectorE as a handful of large ops. prod+reduce are emitted first:
        # they gate the transposes and only need the wL/eps DMAs.
        sp3 = sp[:].rearrange("o (i a) -> o i a", i=I)
        e3 = epsw.rearrange("o (i a) -> o i a", i=I)
        l3 = wloc.rearrange("o (i a) -> o i a", i=I)
        w0 = wsamp[:, 0 : I * D].rearrange("o (i a) -> o i a", i=I)
        wl3 = wl[:].rearrange("o (i a b) -> o i a b", i=I, a=D)
        p3 = tmp2[:].rearrange("o (i a b) -> o i a b", i=I, a=D)
        eb = bass.AP(
            tensor=epsw.tensor,
            offset=epsw.offset,
            ap=[list(p) for p in epsw.ap[:1]] + [[D, I], [0, D], [1, D]],
        )
        # i-halves pipelined: prod/reduce of half 1 run while wL half 2
        # is still in flight
        def half(v, k, idim=1):
            ap = [list(p) for p in v.ap]
            st = ap[idim][0]
            ap[idim] = [st, I // 2]
            return bass.AP(tensor=v.tensor, offset=v.offset + k * (I // 2) * st, ap=ap)

        r3 = redt[:].rearrange("o (i a) -> o i a", i=I)
        for k in range(2):
            nc.vector.tensor_tensor(
                half(p3, k), half(wl3, k), half(eb, k), mybir.AluOpType.mult
            )
            nc.vector.tensor_reduce(
                half(r3, k), half(p3, k), mybir.AxisListType.X, mybir.AluOpType.add
            )
        nc.vector.tensor_tensor(w0, sp3, e3, mybir.AluOpType.mult)
        nc.vector.tensor_add(w0, w0, l3)
        nc.vector.tensor_add(wsamp[:, 0 : I * D], wsamp[:, 0 : I * D], redt[:])
        # duplicate wsamp so transposes can address (half, i) as one run
        nc.vector.tensor_copy(wsamp[:, I * D : 2 * I * D], wsamp[:, 0 : I * D])

        # transpose the 9 taps on the PE into T_a[ich, och] on BOTH partition
        # halves at once (lhsT free dim = 128 spanning the two wsamp copies),
        # packed 5 + 4 into two PSUM banks, then bf16-convert into wts.
        with tc.tile_pool(name="pt", bufs=1, space="PSUM") as ptp:
            ptA = ptp.tile([128, 5 * O], F32, name="ptA")
            ptB = ptp.tile([128, 4 * O], F32, name="ptB")
            for a in range(D):
                w_a = bass.AP(
                    tensor=wsamp[:].tensor,
                    offset=wsamp[:].offset + a,
                    ap=[list(p) for p in wsamp[:].ap[:1]] + [[D, 2 * I]],
                )
                dst_pt = ptA if a < 5 else ptB
                c = a if a < 5 else a - 5
                nc.tensor.matmul(
                    dst_pt[:, c * O : (c + 1) * O],
                    w_a,
                    ident[:],
                    is_transpose=True,
                    start=(c == 0),
                    stop=(c == (4 if a < 5 else 3)),
                    skip_group_check=True,
                )
            nc.vector.tensor_copy(wts[:, 0 : 5 * O], ptA[:])
            nc.vector.tensor_copy(wts[:, 5 * O : 9 * O], ptB[:])

            # bias path, off the conv critical path: bias3 arrives [3, 128]
            # (och duplicated on host); transpose to [128, 3], then
            # bias = loc + eps * softplus(ro). Needed by the first evac
            # (conv start + ~3.4us), not the first matmul.
            bp_ps = ptp.tile([128, 3], F32, name="bp_ps")
            nc.tensor.matmul(
                bp_ps[:], b3p[:], ident[0:3, 0:3], start=True, stop=True
            )
            nc.scalar.activation(b3[:], bp_ps[:], AF.Identity)
            nc.scalar.activation(sp_b[:], b3[:, 1:2], AF.Exp)
            nc.scalar.activation(sp_b[:], sp_b[:], AF.Ln, bias=1.0)
            nc.gpsimd.tensor_mul(sp_b[:], sp_b[:], b3[:, 2:3])
            nc.gpsimd.tensor_add(bias[:], b3[:, 0:1], sp_b[:])

        # ---------------- convolution ---------------------------------------
        # Group = 8 output rows of both images. 4 concurrent 64x64 PE tiles:
        # PE rows half = image, PE cols half = 4-row block parity. Per tap
        # each quadrant runs 2 row-pair matmuls into 2 PSUM banks:
        #   imgA rows 8j+0..1 -> bk0[0:64]   rows 8j+2..3 -> bk1[0:64]
        #   imgA rows 8j+4..5 -> bk0[64:]    rows 8j+6..7 -> bk1[64:]
        #   imgB rows 8j+0..1 -> bk2[0:64]   ... etc (banks 2,3)
        # SBUF out strip: partitions 0:64 = rows 8j+0..3, 64:128 = 8j+4..7;
        # free = [img, group, 4*ww]. Output DMA splits (partition half, img)
        # with 4-row (1792B) descriptors.
        xpa = stack.enter_context(tc.tile_pool(name="xa", bufs=3))
        xpb = stack.enter_context(tc.tile_pool(name="xb", bufs=3))
        op = stack.enter_context(tc.tile_pool(name="ostrip", bufs=o_bufs))
        pp = stack.enter_context(tc.tile_pool(name="acc", bufs=2, space="PSUM"))
        allstrips = []
        for pair in range(nb // 2):
            n0 = 2 * pair
            strips = [(s * rstrip, rstrip) for s in range(nstrips)]
            if pair == nb // 2 - 1 and rstrip >= 16:
                # Taper: split the final strip in half so the kernel does
                # not end waiting on one full-size store.
                h_last = strips.pop()[0]
                strips.append((h_last, rstrip // 2))
                strips.append((h_last + rstrip // 2, rstrip // 2))
            allstrips += [(n0, h0, rout) for h0, rout in strips]

        loaded = {}

        def emit_load(idx):
            n0_, h0_, rout_ = allstrips[idx]
            ng_ = rout_ // 8
            hr_ = rout_ // 2 + 2 if ng_ > 1 else rout_ + 2
            xa = xpa.tile([128, xrows, wpad], BF16, name="xs_a")
            src_ = x_t[n0_ : n0_ + 2, :, h0_ : h0_ + hr_, :].rearrange(
                "n i h w -> (n i) h w"
            )
            nc.sync.dma_start(xa[:, 0:hr_, :], src_)
            xb = None
            if ng_ > 1:
                xb = xpb.tile([128, xrows, wpad], BF16, name="xs_b")
                srcb_ = x_t[
                    n0_ : n0_ + 2, :, h0_ + rout_ // 2 : h0_ + rout_ + 2, :
                ].rearrange("n i h w -> (n i) h w")
                nc.sync.dma_start(xb[:, 0 : rout_ // 2 + 2, :], srcb_)
            loaded[idx] = (xa, xb)

        # loads run 2 strips ahead and are emitted BEFORE the previous
        # strip's stores, so store triggers never head-of-line-block them
        emit_load(0)
        if len(allstrips) > 1:
            emit_load(1)
        for si, (n0, h0, rout) in enumerate(allstrips):
            ngroups = rout // 8
            xs_a, xs_b = loaded.pop(si)
            os_ = op.tile([128, rout * ww], BF16, name="os_")
            for j in range(ngroups):
                bks = [pp.tile([128, 2 * ww], F32, name=f"bk{k}") for k in range(4)]
                if 8 * j >= rout // 2 and ngroups > 1:
                    xs, rbase = xs_b, 8 * j - rout // 2
                else:
                    xs, rbase = xs_a, 8 * j
                for tap in range(D):
                    dy, dx = tap // 3 - 1, tap % 3 - 1
                    st, sp_ = (tap == 0), (tap == D - 1)
                    w_lo = wts[0:O, tap * O : (tap + 1) * O]
                    w_hi = wts[O:128, tap * O : (tap + 1) * O]
                    for rp in range(2):
                        rhs = []
                        for par in range(2):
                            rr = rbase + 4 * par + 2 * rp
                            off = (rr + 1 + dy) * wpad + 1 + dx
                            for half in range(2):
                                base = xs[64 * half : 64 * half + 64]
                                rhs.append(
                                    bass.AP(
                                        tensor=base.tensor,
                                        offset=base.offset + off,
                                        ap=[list(p) for p in base.ap[:1]]
                                        + [[wpad, 2], [1, ww]],
                                    )
                                )
                        # rhs order: [imgA par0, imgB par0, imgA par1, imgB par1]
                        nc.tensor.matmul(
                            bks[rp][0:O], w_lo, rhs[0],
                            start=st, stop=sp_, skip_group_check=True,
                        )
                        nc.tensor.matmul(
                            bks[rp][O:128], w_lo, rhs[2],
                            start=st, stop=sp_, skip_group_check=True,
                        )
                        nc.tensor.matmul(
                            bks[2 + rp][0:O], w_hi, rhs[1],
                            start=st, stop=sp_, skip_group_check=True,
                        )
                        nc.tensor.matmul(
                            bks[2 + rp][O:128], w_hi, rhs[3],
                            start=st, stop=sp_, skip_group_check=True,
                        )
                # evacuate with fused bias add: imgA banks on ScalarE,
                # imgB banks on VectorE; each bank's halves land at the
                # same free offset (parities live on partition halves)
                for rp in range(2):
                    sA = (j * 4 + 2 * rp) * ww
                    nc.scalar.activation(
                        os_[:, sA : sA + 2 * ww],
                        bks[rp][:], AF.Identity, bias=bias[:, 0:1],
                    )
                    sB = ((ngroups + j) * 4 + 2 * rp) * ww
                    nc.vector.tensor_scalar_add(
                        os_[:, sB : sB + 2 * ww],
                        bks[2 + rp][:], bias[:, 0:1],
                    )
            if si + 2 < len(allstrips):
                emit_load(si + 2)
            # store DMAs: (partition half, image), 4-row descriptors. The
            # final two strips store per group so the kernel does not end
            # draining a whole strip of descriptor-rate-bound stores.
            per_group = si >= len(allstrips) - 2
            for jg in range(ngroups) if per_group else [None]:
                for img in range(2):
                    for par in range(2):
                        os_h = os_[64 * par : 64 * par + 64]
                        j0 = jg if per_group else 0
                        ng = 1 if per_group else ngroups
                        src_os = bass.AP(
                            tensor=os_h.tensor,
                            offset=os_h.offset
                            + (img * ngroups + j0) * 4 * ww,
                            ap=[list(p) for p in os_h.ap[:1]]
                            + [[4 * ww, ng], [1, 4 * ww]],
                        )
                        dst = bass.AP(
                            tensor=out_t.tensor,
                            offset=out_t.offset
                            + (n0 + img) * O * hh * ww
                            + (h0 + 8 * j0 + 4 * par) * ww,
                            ap=[[hh * ww, O], [8 * ww, ng], [1, 4 * ww]],
                        )
                        q = nc.gpsimd if par == 0 else nc.sync
                        q.dma_start(dst, src_os)

    nc.compile()
    return nc


_CACHED_NC = None


def _host_inputs(x_shard_padded, weight_loc, weight_L, bias_loc, bias_ro, eps_w, eps_b):
    import ml_dtypes

    bf = ml_dtypes.bfloat16
    wdiag = np.diagonal(weight_L, axis1=-2, axis2=-1).reshape(O, I * D)
    smalls = np.concatenate(
        [wdiag, eps_w.reshape(O, I * D), weight_loc.reshape(O, I * D)], axis=1
    )
    return {
        "x": x_shard_padded,
        "wL": np.ascontiguousarray(
            np.tril(weight_L, -1).reshape(O, I * D * D).astype(bf)
        ),
        "smalls": np.ascontiguousarray(smalls.astype(bf)),
        "ident": np.eye(O, dtype=np.float32),
        "bias3": np.ascontiguousarray(
            np.tile(np.stack([bias_loc, bias_ro, eps_b]).astype(np.float32), (1, 2))
        ),
    }


def _pad_x(x):
    """fp32 [N, I, H, W] -> bf16 [N, I, H+2, W+2] zero-padded halo."""
    import ml_dtypes

    n, i, h, w = x.shape
    xp = np.zeros((n, i, h + 2, w + 2), dtype=ml_dtypes.bfloat16)
    xp[:, :, 1 : h + 1, 1 : w + 1] = x.astype(ml_dtypes.bfloat16)
    return xp


def kernel(x, weight_loc, weight_L, bias_loc, bias_ro, eps_w, eps_b):
    global _CACHED_NC
    from concourse.bass_utils import run_bass_kernel_spmd

    x = np.asarray(x, np.float32)
    nb = x.shape[0] // N_CORES
    if _CACHED_NC is None:
        _CACHED_NC = build_nc(nb=nb)
    nc = _CACHED_NC

    xpad = _pad_x(x)
    in_maps = [
        _host_inputs(
            xpad[c * nb : (c + 1) * nb],
            np.asarray(weight_loc),
            np.asarray(weight_L),
            np.asarray(bias_loc),
            np.asarray(bias_ro),
            np.asarray(eps_w),
            np.asarray(eps_b),
        )
        for c in range(N_CORES)
    ]
    res = run_bass_kernel_spmd(nc, in_maps, list(range(N_CORES)))
    return np.concatenate(
        [res.results[c]["out"].astype(np.float32) for c in range(N_CORES)], axis=0
    )
